# revision 11
# baseline (speedup 1.0000x reference)
"""Trainium2 Bass kernel for nn_DGCRNN (ChebConv K=3 GNN, robot-node output).

Math: the reference returns only node 0 (robot) of the ChebConv output, so
    out = r @ (W0 - W2 + v1[0]*W1 + 2*v2[0]*W2) + cheb_b
        + c1 @ W1 + c2 @ (2*W2)
with v1 = L_hat[0, :], v2 = (L_hat @ L_hat)[0, :] (host-computed from
edge_index), c1 = sum_i v1[i] * h_i, c2 = sum_i v2[i] * h_i over the human
node embeddings h_i, and r the robot embedding.

Sharding: pure data parallel over the batch dim (512 / 8 cores = 64 each).

v2 design (vs the 31.8us fp32 baseline):
- bf16 end-to-end (tolerance is 2e-2; bf16 keeps us ~5e-3): single-pass
  matmuls, 2x DVE throughput, half DMA bytes.
- nodes padded 63 -> 64 so each 512-token slice = 8 batches x 64 nodes fills
  exactly one PSUM bank and keeps bf16 pair alignment; the pad column is
  zeroed via the v-multiplier table.
- PE warm-up: a burst of tiny matmuls during the DMA wait trips the HAM
  clock gate (1.2 -> 2.4 GHz) before real work arrives.
- elementwise split: ACT does relu1 (PSUM->SBUF, free relu), DVE does
  relu2+bias (tensor_scalar add+max), the v-multiply (tensor_tensor) and the
  per-batch reduces; GPSIMD takes two v-multiplies (it cannot reduce along
  the free axis on trn2).
- the final combine is 2 small accumulating matmuls into the robot-term
  PSUM region; output [F, B] is transposed on host.
"""

import numpy as np

B, N, F, HID = 512, 64, 64, 128
ROBOT_DIM, HUMAN_DIM = 9, 5
NCORES = 8
BL = B // NCORES      # 64 batches per core
NH = 63               # real human nodes
NT = 64               # padded nodes per batch
TOK = BL * NT         # 4096 tokens per core
SL = 512              # tokens per slice (one PSUM bank)
NSL = 8               # slices
SB = SL // NT         # 8 batches per slice

PW_W = 644            # packed weight tensor width
NWU = 28              # PE warm-up matmuls

_STATE = {}


def _build_bass():
    import concourse.bass as bass
    from concourse import bacc, mybir

    f32 = mybir.dt.float32
    bf16 = mybir.dt.bfloat16
    AF = mybir.ActivationFunctionType
    ALU = mybir.AluOpType
    AX = mybir.AxisListType

    nc = bacc.Bacc("TRN2", target_bir_lowering=False, debug=False)

    # --- DRAM I/O ---
    d_hTa = nc.dram_tensor("hTa", [HUMAN_DIM + 1, TOK], bf16, kind="ExternalInput").ap()
    d_pw = nc.dram_tensor("pw", [HID, PW_W], bf16, kind="ExternalInput").ap()
    d_pv = nc.dram_tensor("pv", [HID, SL], bf16, kind="ExternalInput").ap()
    d_out = nc.dram_tensor("out", [F, BL], f32, kind="ExternalOutput").ap()

    # --- SBUF ---
    hTa = nc.alloc_sbuf_tensor("hTa_sb", [HUMAN_DIM + 1, TOK], bf16).ap()
    pw = nc.alloc_sbuf_tensor("pw_sb", [HID, PW_W], bf16).ap()
    pv = nc.alloc_sbuf_tensor("pv_sb", [HID, SL], bf16).ap()
    h1 = nc.alloc_sbuf_tensor("h1_sb", [HID, TOK], bf16).ap()
    h2d = nc.alloc_sbuf_tensor("h2d_sb", [HID, TOK], bf16).ap()
    tmp = nc.alloc_sbuf_tensor("tmp_sb", [HID, TOK], bf16).ap()
    c12 = nc.alloc_sbuf_tensor("c12_sb", [HID, BL], bf16).ap()
    r1s = nc.alloc_sbuf_tensor("r1s_sb", [HID, BL], bf16).ap()
    r2s = nc.alloc_sbuf_tensor("r2s_sb", [F + 1, BL], bf16).ap()
    wut = nc.alloc_sbuf_tensor("wut_sb", [HID, 32], bf16).ap()
    o_sb = nc.alloc_sbuf_tensor("o_sb", [F, BL], f32).ap()
    bias_sb = nc.alloc_sbuf_tensor("bias_sb", [HID, 2], f32).ap()

    # pw slices
    wh1a = pw[0:HUMAN_DIM + 1, 0:128]
    wh2d = pw[:, 128:256]
    W12s = pw[:, 256:320]
    bh2d = pw[:, 320:321]
    br2 = pw[0:F, 321:322]
    rTa = pw[0:ROBOT_DIM + 1, 322:386]
    wr1a = pw[0:ROBOT_DIM + 1, 386:514]
    wr2 = pw[:, 514:578]
    Arc = pw[0:F + 1, 578:642]

    # --- PSUM: [128, 4096] = 8 banks of 512 fp32 ---
    ph = nc.alloc_psum_tensor("ph", [HID, 4096], f32).ap()

    def l1b(p):  # L1 psum bank for slice p (banks 0-3)
        return ph[:, 512 * (p % 4): 512 * (p % 4) + SL]

    def l2b(p):  # L2 psum bank for slice p (banks 4-6)
        k = 4 + p % 3
        return ph[:, 512 * k: 512 * k + SL]

    r1p = ph[:, 3072:3136]          # robot L1 out (bank 6, cols 0-64)
    r2p = ph[0:F, 3136:3200]        # robot L2 out (bank 6, cols 64-128)
    po = ph[0:F, 3584:3648]         # final out (bank 7, cols 0-64)
    wup = ph[0:32, 4032:4064]       # warm-up target (bank 7 tail)

    def tmp3(k):  # [128, 16, 64] view of tmp for double-reduce k (k=0..3)
        return bass.AP(tmp.tensor, tmp.offset + 1024 * k,
                       [list(tmp.ap[0]), [NT, 16], [1, NT]])

    def hsl(t, p):  # slice p of a [128, TOK] tensor
        return t[:, SL * p: SL * (p + 1)]

    # --- semaphores ---
    sdh = [nc.alloc_semaphore(f"sdh{c}") for c in range(4)]
    sdw = nc.alloc_semaphore("sdw")
    sdv = nc.alloc_semaphore("sdv")
    sp = nc.alloc_semaphore("sp")
    sa = nc.alloc_semaphore("sa")
    sv = nc.alloc_semaphore("sv")
    sg = nc.alloc_semaphore("sg")
    sq = nc.alloc_semaphore("sq")   # inc-only (out DMA)
    all_sems = sdh + [sdw, sdv, sp, sa, sv, sg]

    # engine for the v-multiply (c_p) per slice: DVE mostly, GPS offload
    C_ON_GPS = {3, 6}

    # sv counter layout: ra1=1 ra2=2, per slice (b)/(c), 4 reduces, 2 copies
    sv_c = {}
    n = 2
    for p in range(NSL):
        n += 1
        sv_c[("b", p)] = n
        if p not in C_ON_GPS:
            n += 1
            sv_c[("c", p)] = n
    for k in range(4):
        n += 1
        sv_c[("d", k)] = n
    sv_c["copy0"] = n + 1
    sv_c["copy1"] = n + 2
    # sg counter: memsets=1, GPS v-multiplies: c3=2, c6=3
    SG_C = {3: 2, 6: 3}

    with nc.Block(no_gpsimd_drain=True) as block:

        @block.sync
        def _(sync):
            sync.dma_start(out=hTa[:, 0:1024], in_=d_hTa[:, 0:1024]).then_inc(sdh[0], 16)
            sync.dma_start(out=pv[:], in_=d_pv[:]).then_inc(sdv, 16)
            sync.dma_start(out=hTa[:, 2048:3072], in_=d_hTa[:, 2048:3072]).then_inc(sdh[2], 16)
            sync.dma_start(out=hTa[:, 3072:4096], in_=d_hTa[:, 3072:4096]).then_inc(sdh[3], 16)
            sync.wait_ge(sv, sv_c["copy0"])
            sync.dma_start(out=d_out[:, 0:32], in_=o_sb[:, 0:32]).then_inc(sq, 16)
            sync.wait_ge(sv, sv_c["copy1"])
            sync.dma_start(out=d_out[:, 32:64], in_=o_sb[:, 32:64]).then_inc(sq, 16)

        @block.tensor
        def _(tensor):
            # warm-up burst: trips the HAM clock gate while DMAs are in flight
            tensor.wait_ge(sg, 1)
            for _ in range(NWU):
                tensor.matmul(wup, wut[:, 0:32], wut[:], start=True, stop=True,
                              skip_group_check=True)
            # sp: 1=rMM1 2=L1_0 3=L1_1 4=rMM2 5=L2_0 6=rfinal 7=L1_2 8=L2_1
            #     9=L1_3 10=L2_2 11=L1_4 12=L2_3 13=L1_5 14=L2_4 15=L1_6
            #     16=L2_5 17=L1_7 18=L2_6 19=L2_7 20=poMM0 21=poMM1
            tensor.wait_ge(sdw, 16)
            tensor.matmul(r1p, wr1a, rTa, start=True, stop=True).then_inc(sp)   # 1
            tensor.wait_ge(sdh[0], 16)
            tensor.matmul(l1b(0), wh1a, hsl(hTa, 0), start=True, stop=True).then_inc(sp)  # 2
            tensor.matmul(l1b(1), wh1a, hsl(hTa, 1), start=True, stop=True).then_inc(sp)  # 3
            tensor.wait_ge(sv, 1)
            tensor.matmul(r2p, wr2, r1s[:], start=True, stop=True).then_inc(sp)  # 4
            tensor.wait_ge(sa, 1)
            tensor.matmul(l2b(0), wh2d, hsl(h1, 0), start=True, stop=True).then_inc(sp)  # 5
            tensor.wait_ge(sv, 2)
            tensor.matmul(po, Arc, r2s[:], start=True, stop=False,
                          skip_group_check=True).then_inc(sp)                    # 6
            tensor.wait_ge(sdh[1], 16)
            tensor.matmul(l1b(2), wh1a, hsl(hTa, 2), start=True, stop=True).then_inc(sp)  # 7
            tensor.wait_ge(sa, 2)
            tensor.matmul(l2b(1), wh2d, hsl(h1, 1), start=True, stop=True).then_inc(sp)  # 8
            tensor.matmul(l1b(3), wh1a, hsl(hTa, 3), start=True, stop=True).then_inc(sp)  # 9
            tensor.wait_ge(sa, 3)
            tensor.matmul(l2b(2), wh2d, hsl(h1, 2), start=True, stop=True).then_inc(sp)  # 10
            tensor.wait_ge(sdh[2], 16)
            tensor.matmul(l1b(4), wh1a, hsl(hTa, 4), start=True, stop=True).then_inc(sp)  # 11
            tensor.wait_ge(sv, sv_c[("b", 0)])
            tensor.matmul(l2b(3), wh2d, hsl(h1, 3), start=True, stop=True).then_inc(sp)  # 12
            tensor.matmul(l1b(5), wh1a, hsl(hTa, 5), start=True, stop=True).then_inc(sp)  # 13
            tensor.wait_ge(sa, 4)
            tensor.wait_ge(sv, sv_c[("b", 1)])
            tensor.matmul(l2b(4), wh2d, hsl(h1, 4), start=True, stop=True).then_inc(sp)  # 14
            tensor.wait_ge(sdh[3], 16)
            tensor.matmul(l1b(6), wh1a, hsl(hTa, 6), start=True, stop=True).then_inc(sp)  # 15
            tensor.wait_ge(sv, sv_c[("b", 2)])
            tensor.matmul(l2b(5), wh2d, hsl(h1, 5), start=True, stop=True).then_inc(sp)  # 16
            tensor.matmul(l1b(7), wh1a, hsl(hTa, 7), start=True, stop=True).then_inc(sp)  # 17
            tensor.wait_ge(sa, 5)
            tensor.wait_ge(sv, sv_c[("b", 3)])
            tensor.matmul(l2b(6), wh2d, hsl(h1, 6), start=True, stop=True).then_inc(sp)  # 18
            tensor.wait_ge(sa, 6)
            tensor.wait_ge(sv, sv_c[("b", 4)])
            tensor.matmul(l2b(7), wh2d, hsl(h1, 7), start=True, stop=True).then_inc(sp)  # 19
            # final combine halves
            tensor.wait_ge(sv, sv_c[("d", 1)])
            tensor.matmul(po[:, 0:32], W12s, c12[:, 0:32], start=False, stop=False,
                          skip_group_check=True).then_inc(sp)                    # 20
            tensor.wait_ge(sv, sv_c[("d", 3)])
            tensor.matmul(po[:, 32:64], W12s, c12[:, 32:64], start=False, stop=True,
                          skip_group_check=True).then_inc(sp)                    # 21

        @block.scalar
        def _(scalar):
            scalar.dma_start(out=pw[:], in_=d_pw[:]).then_inc(sdw, 16)
            scalar.dma_start(out=hTa[:, 1024:2048], in_=d_hTa[:, 1024:2048]).then_inc(sdh[1], 16)
            # sa: 1=a0 2=a1 3=a23 4=a45 5=a6 6=a7
            scalar.wait_ge(sp, 2)
            scalar.activation(hsl(h1, 0), l1b(0), AF.Relu).then_inc(sa)
            scalar.wait_ge(sp, 3)
            scalar.activation(hsl(h1, 1), l1b(1), AF.Relu).then_inc(sa)
            scalar.wait_ge(sp, 9)
            scalar.activation(h1[:, 1024:2048], ph[:, 1024:2048], AF.Relu).then_inc(sa)
            scalar.wait_ge(sp, 13)
            scalar.activation(h1[:, 2048:3072], ph[:, 0:1024], AF.Relu).then_inc(sa)
            scalar.wait_ge(sp, 15)
            scalar.activation(hsl(h1, 6), l1b(6), AF.Relu).then_inc(sa)
            scalar.wait_ge(sp, 17)
            scalar.activation(hsl(h1, 7), l1b(7), AF.Relu).then_inc(sa)

        @block.vector
        def _(vector):
            vector.wait_ge(sdw, 16)
            vector.tensor_copy(bias_sb[:], pw[:, 320:322])  # bf16 -> f32 biases
            vector.wait_ge(sp, 1)
            vector.tensor_scalar_max(r1s[:], r1p, 0.0).then_inc(sv)            # 1
            vector.wait_ge(sp, 4)
            vector.tensor_scalar(r2s[0:F, :], r2p, bias_sb[0:F, 1:2], 0.0,
                                 ALU.add, ALU.max).then_inc(sv)                # 2
            vector.wait_ge(sdv, 16)
            with nc.allow_low_precision("bf16 c12 fine at 2e-2 tolerance"):
                for p in range(NSL):
                    vector.wait_ge(sp, (5, 8, 10, 12, 14, 16, 18, 19)[p])
                    vector.tensor_scalar(hsl(h2d, p), l2b(p), bias_sb[:, 0:1], 0.0,
                                         ALU.add, ALU.max).then_inc(sv)
                    if p not in C_ON_GPS:
                        vector.tensor_tensor(hsl(tmp, p), hsl(h2d, p), pv[:],
                                             ALU.mult).then_inc(sv)
                # reduces: k covers slices 2k, 2k+1; GPS-owned c's need sg
                for k in range(4):
                    for s in (2 * k, 2 * k + 1):
                        if s in C_ON_GPS:
                            vector.wait_ge(sg, SG_C[s])
                    vector.tensor_reduce(c12[:, 16 * k: 16 * k + 16], tmp3(k),
                                         axis=AX.X, op=ALU.add).then_inc(sv)
            vector.wait_ge(sp, 20)
            vector.tensor_copy(o_sb[:, 0:32], po[:, 0:32]).then_inc(sv)
            vector.wait_ge(sp, 21)
            vector.tensor_copy(o_sb[:, 32:64], po[:, 32:64]).then_inc(sv)

        @block.gpsimd
        def _(gpsimd):
            gpsimd.memset(wut[:], 0.25)
            gpsimd.memset(r2s[F:F + 1, :], 1.0).then_inc(sg)                   # 1
            for p in sorted(C_ON_GPS):
                gpsimd.wait_ge(sv, sv_c[("b", p)])
                gpsimd.tensor_tensor(hsl(tmp, p), hsl(h2d, p), pv[:],
                                     ALU.mult).then_inc(sg)

    nc.clear_and_free_semaphores(all_sems)
    nc.compile()
    return nc


def _host_prep(robot_x, human_x, edge_index, wr1_w, wr1_b, wr2_w, wr2_b,
               wh1_w, wh1_b, wh2_w, wh2_b, cheb_w, cheb_b):
    """Graph vectors + packed bf16 weights on host; per-core input maps."""
    import ml_dtypes

    bf16 = ml_dtypes.bfloat16
    f32 = np.float32
    robot_x = np.asarray(robot_x, f32)
    human_x = np.asarray(human_x, f32)
    ei = np.asarray(edge_index)
    src, dst = ei[0].astype(np.int64), ei[1].astype(np.int64)

    deg = np.zeros(N, f32)
    np.add.at(deg, src, f32(1.0))
    dinv = np.where(deg > 0, deg.astype(f32) ** f32(-0.5), f32(0.0)).astype(f32)
    w = -(dinv[src] * dinv[dst])
    L = np.zeros((N, N), f32)
    np.add.at(L, (dst, src), w)
    v1 = L[0].astype(f32)
    v2 = (v1 @ L).astype(f32)

    W0, W1, W2 = (np.asarray(cheb_w, f32)[k] for k in range(3))
    wh1_w = np.asarray(wh1_w, f32); wh1_b = np.asarray(wh1_b, f32)
    wh2_w = np.asarray(wh2_w, f32); wh2_b = np.asarray(wh2_b, f32)
    wr1_w = np.asarray(wr1_w, f32); wr1_b = np.asarray(wr1_b, f32)
    wr2_w = np.asarray(wr2_w, f32); wr2_b = np.asarray(wr2_b, f32)
    cheb_b = np.asarray(cheb_b, f32)

    pw = np.zeros((HID, PW_W), f32)
    pw[0:HUMAN_DIM, 0:128] = wh1_w
    pw[HUMAN_DIM, 0:128] = wh1_b
    pw[:, 128:256] = np.hstack([wh2_w, wh2_w])
    pw[0:F, 256:320] = W1
    pw[F:HID, 256:320] = f32(2.0) * W2
    pw[0:F, 320] = wh2_b
    pw[F:HID, 320] = wh2_b
    pw[0:F, 321] = wr2_b
    pw[0:ROBOT_DIM, 386:514] = wr1_w
    pw[ROBOT_DIM, 386:514] = wr1_b
    pw[:, 514:578] = wr2_w
    pw[0:F, 578:642] = W0 - W2 + v1[0] * W1 + f32(2.0) * v2[0] * W2
    pw[F, 578:642] = cheb_b

    pvm = np.zeros((HID, SL), f32)
    vpat1 = np.zeros(NT, f32); vpat1[0:NH] = v1[1:N]
    vpat2 = np.zeros(NT, f32); vpat2[0:NH] = v2[1:N]
    pvm[0:F, :] = np.tile(vpat1, SB)
    pvm[F:HID, :] = np.tile(vpat2, SB)

    pw_b = pw.astype(bf16)
    pv_b = pvm.astype(bf16)

    in_maps = []
    for c in range(NCORES):
        bs = slice(c * BL, (c + 1) * BL)
        # [6, 4096] token-major: col = b*64 + i; pad col i=63 zero, ones-row 1
        hx = human_x[bs]                            # [BL, 63, 5]
        hT = np.zeros((HUMAN_DIM + 1, BL, NT), f32)
        hT[0:HUMAN_DIM, :, 0:NH] = hx.transpose(2, 0, 1)
        hT[HUMAN_DIM, :, :] = f32(1.0)
        pwc = pw_b.copy()
        pwc[0:ROBOT_DIM, 322:386] = robot_x[bs, 0, :].T.astype(bf16)
        pwc[ROBOT_DIM, 322:386] = f32(1.0)
        in_maps.append({
            "hTa": np.ascontiguousarray(hT.reshape(HUMAN_DIM + 1, TOK).astype(bf16)),
            "pw": pwc,
            "pv": pv_b,
        })
    return in_maps


def run(inputs, trace=False, tmpdir=None):
    """Run the Bass kernel on 8 cores. Returns (full_output, BassKernelResults)."""
    from concourse.bass_utils import run_bass_kernel_spmd

    if "nc" not in _STATE:
        _STATE["nc"] = _build_bass()
    nc = _STATE["nc"]

    in_maps = _host_prep(**inputs)
    res = run_bass_kernel_spmd(
        nc, in_maps, list(range(NCORES)), trace=trace, tmpdir=tmpdir
    )
    out = np.concatenate(
        [np.asarray(res.results[c]["out"], np.float32).T for c in range(NCORES)],
        axis=0,
    )
    return out, res


def kernel(**inputs) -> np.ndarray:
    out, _ = run(inputs, trace=False)
    return out
